# revision 59
# baseline (speedup 1.0000x reference)
"""Trainium2 Bass kernel for a batched HGNN layer.

Per batch b (N=4096 nodes, E=2048 hyperedges, C=128 channels):
    De = sum_n H[n,e] + eps                 (hyperedge degrees)
    Dv = sum_e H[n,e] + eps                 (node degrees)
    s  = 1/sqrt(Dv)
    out = ((H @ ((H^T @ (x * s)) / De)) * s) @ W^T + b

Sharding: batch dim B=8, one batch per NeuronCore (data parallel, no
cross-core communication). Inside a core:

  pass 1 (streams H once from HBM in 8 superchunks of 512 rows,
          software-pipelined, fp32->bf16 cast done by the SWDGE DMA
          itself so ACT/DVE only do copies/reductions):
    - Dv row-sums via DVE reduce on the bf16 tile
    - out2T[c,e] = (x*s)^T @ H accumulated in PSUM (PE, bf16)
    - H^T built with PE transposes, staged via PSUM, copied to a
      16 MB bf16 SBUF cache with plain ACT/DVE copies (no accum_out)
    - De col-sums via grouped DVE reduces over the fresh H^T slices
  interlude (fused into pass 2's first block-pair stream):
    - out3[e,c] = transpose(out2T) * (1/De)
  pass 2 (H^T streamed from SBUF, no HBM traffic; 4 groups of 2
          column blocks so each group's epilogue overlaps the next
          group's matmul stream):
    - out4T[c,n] = out3^T @ H^T  (PE, bf16)
    - out[n,co] = (out4T_tile^T @ W^T)*s + b  (PE bf16, DVE epilogue)

HBM traffic per core = 32 MB (H) + 2 MB (x) + 2 MB (out) ~= 36 MB,
i.e. the memory roofline for this problem.
"""
import os
import sys

import numpy as np

for _p in ("/opt/trn_rl_repo", "/root/.axon_site/_ro/trn_rl_repo"):
    if os.path.isdir(_p) and _p not in sys.path:
        sys.path.append(_p)

B, N, E, C = 8, 4096, 2048, 128
SC = 4                      # subchunks (128 rows) per superchunk
KHT = 130                   # H^T cache block: 128 cols + De partial + pad
                            # (130 keeps each PSUM transpose dest 4B-aligned)
NSUPER = N // (128 * SC)    # 8 superchunks in pass 1
NCHUNKS = N // 128          # 32 row chunks
ETILES = E // 128           # 16 hyperedge tiles
EPS = 1e-6

_CACHE = {}


def _build_nc():
    from contextlib import ExitStack

    import concourse.tile as tile
    from concourse import bacc, mybir

    F32 = mybir.dt.float32
    BF16 = mybir.dt.bfloat16
    X = mybir.AxisListType.X
    XY = mybir.AxisListType.XY
    COPY = mybir.ActivationFunctionType.Copy
    ADD = mybir.AluOpType.add

    nc = bacc.Bacc("TRN2", target_bir_lowering=False, debug=False)

    H_d = nc.dram_tensor("H", [N, E], F32, kind="ExternalInput")
    x_d = nc.dram_tensor("x", [N, C], F32, kind="ExternalInput")
    W_d = nc.dram_tensor("Wt", [C, C], F32, kind="ExternalInput")
    b_d = nc.dram_tensor("b", [1, C], F32, kind="ExternalInput")
    out_d = nc.dram_tensor("out", [N, C], F32, kind="ExternalOutput")

    H_ap, x_ap, out_ap = H_d.ap(), x_d.ap(), out_d.ap()

    with tile.TileContext(nc) as tc:
        with ExitStack() as ctx:
            const = ctx.enter_context(tc.tile_pool(name="const", bufs=1))
            h32p = ctx.enter_context(tc.tile_pool(name="h32", bufs=2))
            h16p = ctx.enter_context(tc.tile_pool(name="h16", bufs=6))
            xpool = ctx.enter_context(tc.tile_pool(name="xp", bufs=2))
            spool = ctx.enter_context(tc.tile_pool(name="sp", bufs=2))
            opool = ctx.enter_context(tc.tile_pool(name="op", bufs=2))
            psT_cm = tc.tile_pool(name="psT", bufs=4, space="PSUM")
            psT = psT_cm.__enter__()
            psA_cm = tc.tile_pool(name="psA", bufs=1, space="PSUM")
            psA = psA_cm.__enter__()

            # --- constants -------------------------------------------------
            # Extended identity [I | 1 1]: transposing with it makes columns
            # 128/129 of each PE transpose the partition-sum of the input
            # tile, i.e. a free De (hyperedge-degree) partial -- no
            # accum_out needed on the staging copies.
            ident16 = const.tile([128, 128], BF16)
            nc.vector.memset(ident16[:], 1.0)
            nc.gpsimd.affine_select(
                ident16[:], ident16[:], pattern=[[-1, 128]], base=0,
                channel_multiplier=1, compare_op=mybir.AluOpType.is_equal,
                fill=0.0,
            )
            ones_n = const.tile([128, 1], BF16)
            nc.vector.memset(ones_n[:], 1.0)

            # --- persistent state ------------------------------------------
            HT = const.tile([128, ETILES * N], BF16)     # H^T cache, 128 KB/part
            out3 = const.tile([128, ETILES * 128], BF16)  # (H^T xs)/De, [e, c]
            Isd = const.tile([128, NCHUNKS], F32)        # 1/sqrt(Dv)
            DvRaw = const.tile([128, NCHUNKS], F32)
            DeP2 = const.tile([128, ETILES * NSUPER], F32)  # De partials
            RecDe = const.tile([128, ETILES], F32)

            out2T_ps = psA.tile([128, E], F32)           # 4 PSUM banks

            HT3 = HT[:].rearrange("p (j n) -> p j n", j=ETILES)
            DeP3 = DeP2[:].rearrange("p (j i) -> p j i", j=ETILES)

            # --- pass 1 (software pipelined) -------------------------------
            def load(i):
                """DMA superchunk i (fp32) on the sync HWDGE ring.

                All H chunks go on nc.sync: the sync engine is otherwise
                idle, so triggers issue immediately.  (nc.scalar triggers
                sit in the busy ACT queue and stall the stream.)  2 MiB per
                call (2 row-chunks) for better DMA efficiency and fewer
                completion gaps.
                """
                h32s = []
                for h in range(SC // 2):
                    h32 = h32p.tile([128, 2, E], F32, tag="h32")
                    r0 = (i * SC + 2 * h) * 128
                    nc.sync.dma_start(
                        h32[:],
                        H_ap[r0:r0 + 256, :].rearrange("(t p) e -> p t e",
                                                       p=128),
                    )
                    h32s.append(h32[:, 0, :])
                    h32s.append(h32[:, 1, :])
                return h32s

            def compute(i, h32s):
                x_t = xpool.tile([128, SC, C], F32, tag="x")
                nc.gpsimd.dma_start(
                    x_t[:],
                    x_ap[i * SC * 128:(i + 1) * SC * 128, :].rearrange(
                        "(t p) c -> p t c", p=128
                    ),
                )
                # fp32->bf16 casts with Dv row-sums fused via accum_out,
                # alternating ACT/DVE (explicit TensorReduce has no fast
                # DVE mode, so fusion is the only affordable reduction).
                h16s = []
                for t in range(SC):
                    ci = i * SC + t
                    h16 = h16p.tile([128, E], BF16, tag="h16",
                                    name=f"h16_{i}_{t}")
                    if t % 2 == 0:
                        nc.scalar.activation(
                            h16[:], h32s[t], COPY,
                            accum_out=DvRaw[:, ci:ci + 1],
                        )
                    else:
                        nc.vector.tensor_scalar(
                            h16[:], h32s[t], 0.0, None, ADD, ADD,
                            accum_out=DvRaw[:, ci:ci + 1],
                        )
                    h16s.append(h16)
                rec = spool.tile([128, SC], F32, tag="rec")
                nc.gpsimd.tensor_scalar_add(
                    rec[:], DvRaw[:, i * SC:(i + 1) * SC], EPS
                )
                nc.vector.reciprocal(rec[:], rec[:])
                nc.scalar.sqrt(Isd[:, i * SC:(i + 1) * SC], rec[:])

                # xs scaling on the otherwise-idle GPSIMD
                xs16 = xpool.tile([128, SC, C], BF16, tag="xs")
                for t in range(SC):
                    ci = i * SC + t
                    nc.gpsimd.tensor_scalar_mul(
                        xs16[:, t, :], x_t[:, t, :], Isd[:, ci:ci + 1]
                    )

                for t in range(SC):
                    for s in range(4):
                        nc.tensor.matmul(
                            out2T_ps[:, s * 512:(s + 1) * 512],
                            xs16[:, t, :],
                            h16s[t][:, s * 512:(s + 1) * 512],
                            start=(i == 0 and t == 0),
                            stop=(i == NSUPER - 1 and t == SC - 1),
                        )

                # De partials for this superchunk on the PE: tiny Nf=1
                # ones-matmuls accumulated over the 4 chunks per e-tile.
                # Keeps the staging copies accum_out-free (the DVE/ACT
                # fused-reduce tax was the pass-1 bottleneck).
                deps = psT.tile([128, ETILES], F32, tag="stg",
                                name=f"deps_{i}")
                for j in range(ETILES):
                    stg = psT.tile([128, SC * 128], BF16, tag="stg")
                    for t in range(SC):
                        nc.tensor.transpose(
                            stg[:, t * 128:(t + 1) * 128],
                            h16s[t][:, j * 128:(j + 1) * 128],
                            ident16[:],
                        )
                    for t in range(SC):
                        nc.tensor.matmul(
                            deps[:, j:j + 1],
                            h16s[t][:, j * 128:(j + 1) * 128],
                            ones_n[:], start=(t == 0), stop=(t == SC - 1),
                        )
                    dest = HT3[:, j, i * SC * 128:(i + 1) * SC * 128]
                    # 7 copies on ACT, 9 on DVE (ACT also carries 2 casts
                    # and is the tighter engine)
                    if j % 2 == 0 and j != 14:
                        nc.scalar.copy(dest, stg[:])
                    else:
                        nc.vector.tensor_copy(dest, stg[:])
                nc.scalar.copy(DeP3[:, :, i:i + 1], deps[:].rearrange(
                    "p (j o) -> p j o", o=1))

            # W / b prep first: gpsimd DMAs + PE/DVE are idle at startup,
            # and the H stream on the sync ring is not delayed by these.
            wt32 = spool.tile([128, 128], F32, tag="wt32")
            nc.gpsimd.dma_start(wt32[:], W_d.ap())
            wt16 = const.tile([128, 128], BF16)          # W^T: [c_in, c_out]
            nc.vector.tensor_copy(wt16[:], wt32[:])

            b_sb = const.tile([1, 128], F32)
            nc.gpsimd.dma_start(b_sb[:], b_d.ap())
            ones1 = const.tile([1, 128], F32)
            nc.vector.memset(ones1[:], 1.0)
            bb_ps = psT.tile([128, 128], F32, tag="stg")
            nc.tensor.matmul(bb_ps[:], ones1[:], b_sb[:], start=True, stop=True)
            b_bcast = const.tile([128, 128], F32)        # b replicated per row
            nc.scalar.copy(b_bcast[:], bb_ps[:])

            h32s_cur = load(0)
            for i in range(NSUPER):
                h32s_next = load(i + 1) if i + 1 < NSUPER else None
                compute(i, h32s_cur)
                h32s_cur = h32s_next

            # --- interlude: De totals, copy out2 out of PSUM ---------------
            nc.vector.reduce_sum(RecDe[:], DeP3[:, :, :], axis=X)
            nc.vector.tensor_scalar_add(RecDe[:], RecDe[:], EPS)
            nc.vector.reciprocal(RecDe[:], RecDe[:])

            # out2T lands (bf16) in out3's buffer; each e-tile is then
            # transposed out and the scaled result overwrites it in place.
            nc.scalar.copy(out3[:, 0:1024], out2T_ps[:, 0:1024])
            nc.vector.tensor_copy(out3[:, 1024:2048], out2T_ps[:, 1024:2048])

            psA_cm.__exit__(None, None, None)

            # --- pass 2: 4 groups of column blocks (3+2+2+1) ---------------
            # Group 0's matmul stream is interleaved with the out3 build
            # (transpose + 1/De scale per e-tile); each group's epilogue is
            # emitted after the NEXT group's stream so PE stays dense, and
            # the last group is a single block to minimize the exposed tail.
            GROUPS = [[0, 1], [2, 3], [4, 5], [6], [7]]
            psB_cm = tc.tile_pool(name="psB", bufs=4, space="PSUM")
            psB = psB_cm.__enter__()

            o4 = {}
            for grp in GROUPS:
                for blk in grp:
                    o4[blk] = psB.tile([128, 512], F32, tag="o4",
                                       name=f"o4_{blk}")

            def jstream(g):
                for j in range(ETILES):
                    for blk in GROUPS[g]:
                        nc.tensor.matmul(
                            o4[blk][:],
                            out3[:, j * 128:(j + 1) * 128],
                            HT[:, j * N + blk * 512:j * N + (blk + 1) * 512],
                            start=(j == 0), stop=(j == ETILES - 1),
                        )

            # group 0 + out3 build, interleaved per e-tile
            for j in range(ETILES):
                t2 = psT.tile([128, 128], BF16, tag="stg")
                nc.tensor.transpose(
                    t2[:], out3[:, j * 128:(j + 1) * 128], ident16[:]
                )
                if j % 2 == 0:
                    nc.scalar.mul(
                        out3[:, j * 128:(j + 1) * 128], t2[:],
                        RecDe[:, j:j + 1]
                    )
                else:
                    nc.vector.tensor_scalar_mul(
                        out3[:, j * 128:(j + 1) * 128], t2[:],
                        RecDe[:, j:j + 1]
                    )
                for blk in GROUPS[0]:
                    nc.tensor.matmul(
                        o4[blk][:],
                        out3[:, j * 128:(j + 1) * 128],
                        HT[:, j * N + blk * 512:j * N + (blk + 1) * 512],
                        start=(j == 0), stop=(j == ETILES - 1),
                    )

            def epilogue(g):
                for blk in GROUPS[g]:
                    # all o4sb copies on ACT: keeps them out of the DVE
                    # queue so lp matmuls never wait behind queued stt's
                    o4sb = opool.tile([128, 512], BF16, tag="o4sb")
                    nc.scalar.copy(o4sb[:], o4[blk][:])
                    obig = opool.tile([128, 4, C], F32, tag="obig",
                                      name=f"obig{blk}")
                    for t in range(4):
                        idx = blk * 4 + t
                        lp = psT.tile([128, 128], F32, tag="stg")
                        nc.tensor.matmul(
                            lp[:], o4sb[:, t * 128:(t + 1) * 128], wt16[:],
                            start=True, stop=True,
                        )
                        nc.vector.scalar_tensor_tensor(
                            obig[:, t, :], lp[:], Isd[:, idx:idx + 1],
                            b_bcast[:],
                            mybir.AluOpType.mult, mybir.AluOpType.add,
                        )
                    nc.gpsimd.dma_start(
                        out_ap[blk * 512:(blk + 1) * 512, :].rearrange(
                            "(t p) c -> p t c", p=128
                        ),
                        obig[:],
                    )

            jstream(1)
            epilogue(0)
            jstream(2)
            epilogue(1)
            jstream(3)
            epilogue(2)
            jstream(4)
            epilogue(3)
            epilogue(4)

            psB_cm.__exit__(None, None, None)
            psT_cm.__exit__(None, None, None)

    nc.compile()
    return nc


def _get_nc():
    if "nc" not in _CACHE:
        _CACHE["nc"] = _build_nc()
    return _CACHE["nc"]


def kernel(x, H, W, b):
    from concourse.bass_utils import run_bass_kernel_spmd

    nc = _get_nc()
    x = np.ascontiguousarray(x, dtype=np.float32)
    H = np.ascontiguousarray(H, dtype=np.float32)
    W = np.ascontiguousarray(W, dtype=np.float32)
    b2 = np.ascontiguousarray(b, dtype=np.float32).reshape(1, C)
    Wt = np.ascontiguousarray(W.T)
    in_maps = [
        {"x": x[c], "H": H[c], "Wt": Wt, "b": b2} for c in range(B)
    ]
    res = run_bass_kernel_spmd(nc, in_maps, core_ids=list(range(B)))
    return np.stack([res.results[c]["out"] for c in range(B)], axis=0)


# revision 62
# speedup vs baseline: 1.1656x; 1.1656x over previous
"""Trainium2 Bass kernel for a batched HGNN layer.

Per batch b (N=4096 nodes, E=2048 hyperedges, C=128 channels):
    De = sum_n H[n,e] + eps                 (hyperedge degrees)
    Dv = sum_e H[n,e] + eps                 (node degrees)
    s  = 1/sqrt(Dv)
    out = ((H @ ((H^T @ (x * s)) / De)) * s) @ W^T + b

Sharding: batch dim B=8, one batch per NeuronCore (data parallel, no
cross-core communication). Inside a core:

  pass 1 (streams H once from HBM in 8 superchunks of 512 rows,
          software-pipelined, fp32->bf16 cast done by the SWDGE DMA
          itself so ACT/DVE only do copies/reductions):
    - Dv row-sums via DVE reduce on the bf16 tile
    - out2T[c,e] = (x*s)^T @ H accumulated in PSUM (PE, bf16)
    - H^T built with PE transposes, staged via PSUM, copied to a
      16 MB bf16 SBUF cache with plain ACT/DVE copies (no accum_out)
    - De col-sums via grouped DVE reduces over the fresh H^T slices
  interlude (fused into pass 2's first block-pair stream):
    - out3[e,c] = transpose(out2T) * (1/De)
  pass 2 (H^T streamed from SBUF, no HBM traffic; 4 groups of 2
          column blocks so each group's epilogue overlaps the next
          group's matmul stream):
    - out4T[c,n] = out3^T @ H^T  (PE, bf16)
    - out[n,co] = (out4T_tile^T @ W^T)*s + b  (PE bf16, DVE epilogue)

HBM traffic per core = 32 MB (H) + 2 MB (x) + 2 MB (out) ~= 36 MB,
i.e. the memory roofline for this problem.
"""
import os
import sys

import numpy as np

for _p in ("/opt/trn_rl_repo", "/root/.axon_site/_ro/trn_rl_repo"):
    if os.path.isdir(_p) and _p not in sys.path:
        sys.path.append(_p)

B, N, E, C = 8, 4096, 2048, 128
SC = 4                      # subchunks (128 rows) per superchunk
KHT = 130                   # H^T cache block: 128 cols + De partial + pad
                            # (130 keeps each PSUM transpose dest 4B-aligned)
NSUPER = N // (128 * SC)    # 8 superchunks in pass 1
NCHUNKS = N // 128          # 32 row chunks
ETILES = E // 128           # 16 hyperedge tiles
EPS = 1e-6

_CACHE = {}


def _build_nc():
    from contextlib import ExitStack

    import concourse.tile as tile
    from concourse import bacc, mybir

    F32 = mybir.dt.float32
    BF16 = mybir.dt.bfloat16
    X = mybir.AxisListType.X
    XY = mybir.AxisListType.XY
    COPY = mybir.ActivationFunctionType.Copy
    ADD = mybir.AluOpType.add

    nc = bacc.Bacc("TRN2", target_bir_lowering=False, debug=False)

    H_d = nc.dram_tensor("H", [N, E], F32, kind="ExternalInput")
    x_d = nc.dram_tensor("x", [N, C], F32, kind="ExternalInput")
    W_d = nc.dram_tensor("Wt", [C, C], F32, kind="ExternalInput")
    b_d = nc.dram_tensor("b", [1, C], F32, kind="ExternalInput")
    out_d = nc.dram_tensor("out", [N, C], F32, kind="ExternalOutput")

    H_ap, x_ap, out_ap = H_d.ap(), x_d.ap(), out_d.ap()

    with tile.TileContext(nc) as tc:
        with ExitStack() as ctx:
            const = ctx.enter_context(tc.tile_pool(name="const", bufs=1))
            h32p = ctx.enter_context(tc.tile_pool(name="h32", bufs=2))
            h16p = ctx.enter_context(tc.tile_pool(name="h16", bufs=6))
            xpool = ctx.enter_context(tc.tile_pool(name="xp", bufs=2))
            spool = ctx.enter_context(tc.tile_pool(name="sp", bufs=2))
            opool = ctx.enter_context(tc.tile_pool(name="op", bufs=2))
            psT_cm = tc.tile_pool(name="psT", bufs=4, space="PSUM")
            psT = psT_cm.__enter__()
            psA_cm = tc.tile_pool(name="psA", bufs=1, space="PSUM")
            psA = psA_cm.__enter__()

            # --- constants -------------------------------------------------
            # Extended identity [I | 1 1]: transposing with it makes columns
            # 128/129 of each PE transpose the partition-sum of the input
            # tile, i.e. a free De (hyperedge-degree) partial -- no
            # accum_out needed on the staging copies.
            ident16 = const.tile([128, 128], BF16)
            nc.vector.memset(ident16[:], 1.0)
            nc.gpsimd.affine_select(
                ident16[:], ident16[:], pattern=[[-1, 128]], base=0,
                channel_multiplier=1, compare_op=mybir.AluOpType.is_equal,
                fill=0.0,
            )
            ones_n = const.tile([128, 1], BF16)
            nc.vector.memset(ones_n[:], 1.0)

            # --- persistent state ------------------------------------------
            HT = const.tile([128, ETILES * N], BF16)     # H^T cache, 128 KB/part
            out3 = const.tile([128, ETILES * 128], BF16)  # (H^T xs)/De, [e, c]
            Isd = const.tile([128, NCHUNKS], F32)        # 1/sqrt(Dv)
            DvRaw = const.tile([128, NCHUNKS], F32)
            DeP2 = const.tile([128, ETILES * NSUPER], F32)  # De partials
            RecDe = const.tile([128, ETILES], F32)

            out2T_ps = psA.tile([128, E], F32)           # 4 PSUM banks

            HT3 = HT[:].rearrange("p (j n) -> p j n", j=ETILES)
            DeP3 = DeP2[:].rearrange("p (j i) -> p j i", j=ETILES)

            # --- pass 1 (software pipelined) -------------------------------
            def load(i):
                """DMA superchunk i (fp32) on the sync HWDGE ring.

                All H chunks go on nc.sync: the sync engine is otherwise
                idle, so triggers issue immediately.  (nc.scalar triggers
                sit in the busy ACT queue and stall the stream.)  2 MiB per
                call (2 row-chunks) for better DMA efficiency and fewer
                completion gaps.
                """
                h32s = []
                for h in range(SC // 2):
                    h32 = h32p.tile([128, 2, E], F32, tag="h32")
                    r0 = (i * SC + 2 * h) * 128
                    nc.sync.dma_start(
                        h32[:],
                        H_ap[r0:r0 + 256, :].rearrange("(t p) e -> p t e",
                                                       p=128),
                    )
                    h32s.append(h32[:, 0, :])
                    h32s.append(h32[:, 1, :])
                return h32s

            def compute(i, h32s):
                x_t = xpool.tile([128, SC, C], F32, tag="x")
                nc.gpsimd.dma_start(
                    x_t[:],
                    x_ap[i * SC * 128:(i + 1) * SC * 128, :].rearrange(
                        "(t p) c -> p t c", p=128
                    ),
                )
                # fp32->bf16 casts with Dv row-sums fused via accum_out,
                # alternating ACT/DVE (explicit TensorReduce has no fast
                # DVE mode, so fusion is the only affordable reduction).
                h16s = []
                for t in range(SC):
                    ci = i * SC + t
                    h16 = h16p.tile([128, E], BF16, tag="h16",
                                    name=f"h16_{i}_{t}")
                    if t % 2 == 0:
                        nc.scalar.activation(
                            h16[:], h32s[t], COPY,
                            accum_out=DvRaw[:, ci:ci + 1],
                        )
                    else:
                        nc.vector.tensor_scalar(
                            h16[:], h32s[t], 0.0, None, ADD, ADD,
                            accum_out=DvRaw[:, ci:ci + 1],
                        )
                    h16s.append(h16)
                rec = spool.tile([128, SC], F32, tag="rec")
                nc.vector.tensor_scalar_add(
                    rec[:], DvRaw[:, i * SC:(i + 1) * SC], EPS
                )
                nc.vector.reciprocal(rec[:], rec[:])
                nc.scalar.sqrt(Isd[:, i * SC:(i + 1) * SC], rec[:])

                xs16 = xpool.tile([128, SC, C], BF16, tag="xs")
                for t in range(SC):
                    ci = i * SC + t
                    if t % 2 == 0:
                        nc.scalar.mul(
                            xs16[:, t, :], x_t[:, t, :], Isd[:, ci:ci + 1]
                        )
                    else:
                        nc.vector.tensor_scalar_mul(
                            xs16[:, t, :], x_t[:, t, :], Isd[:, ci:ci + 1]
                        )

                for t in range(SC):
                    for s in range(4):
                        nc.tensor.matmul(
                            out2T_ps[:, s * 512:(s + 1) * 512],
                            xs16[:, t, :],
                            h16s[t][:, s * 512:(s + 1) * 512],
                            start=(i == 0 and t == 0),
                            stop=(i == NSUPER - 1 and t == SC - 1),
                        )

                # De partials for this superchunk on the PE: tiny Nf=1
                # ones-matmuls accumulated over the 4 chunks per e-tile.
                # Keeps the staging copies accum_out-free (the DVE/ACT
                # fused-reduce tax was the pass-1 bottleneck).
                deps = psT.tile([128, ETILES], F32, tag="stg",
                                name=f"deps_{i}")
                for j in range(ETILES):
                    stg = psT.tile([128, SC * 128], BF16, tag="stg")
                    for t in range(SC):
                        nc.tensor.transpose(
                            stg[:, t * 128:(t + 1) * 128],
                            h16s[t][:, j * 128:(j + 1) * 128],
                            ident16[:],
                        )
                    for t in range(SC):
                        nc.tensor.matmul(
                            deps[:, j:j + 1],
                            h16s[t][:, j * 128:(j + 1) * 128],
                            ones_n[:], start=(t == 0), stop=(t == SC - 1),
                        )
                    dest = HT3[:, j, i * SC * 128:(i + 1) * SC * 128]
                    # 6 copies on ACT, 10 on DVE (ACT also carries 2 casts
                    # and is the tighter engine)
                    if j % 2 == 0 and j not in (6, 14):
                        nc.scalar.copy(dest, stg[:])
                    else:
                        nc.vector.tensor_copy(dest, stg[:])
                nc.scalar.copy(DeP3[:, :, i:i + 1], deps[:].rearrange(
                    "p (j o) -> p j o", o=1))

            # W / b prep first: gpsimd DMAs + PE/DVE are idle at startup,
            # and the H stream on the sync ring is not delayed by these.
            wt32 = spool.tile([128, 128], F32, tag="wt32")
            nc.gpsimd.dma_start(wt32[:], W_d.ap())
            wt16 = const.tile([128, 128], BF16)          # W^T: [c_in, c_out]
            nc.vector.tensor_copy(wt16[:], wt32[:])

            b_sb = const.tile([1, 128], F32)
            nc.gpsimd.dma_start(b_sb[:], b_d.ap())
            ones1 = const.tile([1, 128], F32)
            nc.vector.memset(ones1[:], 1.0)
            bb_ps = psT.tile([128, 128], F32, tag="stg")
            nc.tensor.matmul(bb_ps[:], ones1[:], b_sb[:], start=True, stop=True)
            b_bcast = const.tile([128, 128], F32)        # b replicated per row
            nc.scalar.copy(b_bcast[:], bb_ps[:])

            h32s_cur = load(0)
            for i in range(NSUPER):
                h32s_next = load(i + 1) if i + 1 < NSUPER else None
                compute(i, h32s_cur)
                h32s_cur = h32s_next

            # --- interlude: De totals, copy out2 out of PSUM ---------------
            nc.vector.reduce_sum(RecDe[:], DeP3[:, :, :], axis=X)
            nc.vector.tensor_scalar_add(RecDe[:], RecDe[:], EPS)
            nc.vector.reciprocal(RecDe[:], RecDe[:])

            # out2T lands (bf16) in out3's buffer; each e-tile is then
            # transposed out and the scaled result overwrites it in place.
            # Four half-size copies so the first pass-2 transpose (which
            # only needs cols 0:512) starts sooner and the PE re-ramps.
            nc.scalar.copy(out3[:, 0:512], out2T_ps[:, 0:512])
            nc.vector.tensor_copy(out3[:, 1024:1536], out2T_ps[:, 1024:1536])
            nc.scalar.copy(out3[:, 512:1024], out2T_ps[:, 512:1024])
            nc.vector.tensor_copy(out3[:, 1536:2048], out2T_ps[:, 1536:2048])

            psA_cm.__exit__(None, None, None)

            # --- pass 2: 4 groups of column blocks (3+2+2+1) ---------------
            # Group 0's matmul stream is interleaved with the out3 build
            # (transpose + 1/De scale per e-tile); each group's epilogue is
            # emitted after the NEXT group's stream so PE stays dense, and
            # the last group is a single block to minimize the exposed tail.
            GROUPS = [[0, 1], [2, 3], [4, 5], [6], [7]]
            psB_cm = tc.tile_pool(name="psB", bufs=4, space="PSUM")
            psB = psB_cm.__enter__()

            o4 = {}
            for grp in GROUPS:
                for blk in grp:
                    o4[blk] = psB.tile([128, 512], F32, tag="o4",
                                       name=f"o4_{blk}")

            def jstream(g):
                for j in range(ETILES):
                    for blk in GROUPS[g]:
                        nc.tensor.matmul(
                            o4[blk][:],
                            out3[:, j * 128:(j + 1) * 128],
                            HT[:, j * N + blk * 512:j * N + (blk + 1) * 512],
                            start=(j == 0), stop=(j == ETILES - 1),
                        )

            # group 0 + out3 build, interleaved per e-tile
            for j in range(ETILES):
                t2 = psT.tile([128, 128], BF16, tag="stg")
                nc.tensor.transpose(
                    t2[:], out3[:, j * 128:(j + 1) * 128], ident16[:]
                )
                if j % 2 == 0:
                    nc.scalar.mul(
                        out3[:, j * 128:(j + 1) * 128], t2[:],
                        RecDe[:, j:j + 1]
                    )
                else:
                    nc.vector.tensor_scalar_mul(
                        out3[:, j * 128:(j + 1) * 128], t2[:],
                        RecDe[:, j:j + 1]
                    )
                for blk in GROUPS[0]:
                    nc.tensor.matmul(
                        o4[blk][:],
                        out3[:, j * 128:(j + 1) * 128],
                        HT[:, j * N + blk * 512:j * N + (blk + 1) * 512],
                        start=(j == 0), stop=(j == ETILES - 1),
                    )

            def epilogue(g):
                for blk in GROUPS[g]:
                    # all o4sb copies on ACT: keeps them out of the DVE
                    # queue so lp matmuls never wait behind queued stt's
                    o4sb = opool.tile([128, 512], BF16, tag="o4sb")
                    nc.scalar.copy(o4sb[:], o4[blk][:])
                    obig = opool.tile([128, 4, C], F32, tag="obig",
                                      name=f"obig{blk}")
                    for t in range(4):
                        idx = blk * 4 + t
                        lp = psT.tile([128, 128], F32, tag="stg")
                        nc.tensor.matmul(
                            lp[:], o4sb[:, t * 128:(t + 1) * 128], wt16[:],
                            start=True, stop=True,
                        )
                        nc.vector.scalar_tensor_tensor(
                            obig[:, t, :], lp[:], Isd[:, idx:idx + 1],
                            b_bcast[:],
                            mybir.AluOpType.mult, mybir.AluOpType.add,
                        )
                    nc.gpsimd.dma_start(
                        out_ap[blk * 512:(blk + 1) * 512, :].rearrange(
                            "(t p) c -> p t c", p=128
                        ),
                        obig[:],
                    )

            jstream(1)
            epilogue(0)
            jstream(2)
            epilogue(1)
            jstream(3)
            epilogue(2)
            jstream(4)
            epilogue(3)
            epilogue(4)

            psB_cm.__exit__(None, None, None)
            psT_cm.__exit__(None, None, None)

    nc.compile()
    return nc


def _get_nc():
    if "nc" not in _CACHE:
        _CACHE["nc"] = _build_nc()
    return _CACHE["nc"]


def kernel(x, H, W, b):
    from concourse.bass_utils import run_bass_kernel_spmd

    nc = _get_nc()
    x = np.ascontiguousarray(x, dtype=np.float32)
    H = np.ascontiguousarray(H, dtype=np.float32)
    W = np.ascontiguousarray(W, dtype=np.float32)
    b2 = np.ascontiguousarray(b, dtype=np.float32).reshape(1, C)
    Wt = np.ascontiguousarray(W.T)
    in_maps = [
        {"x": x[c], "H": H[c], "Wt": Wt, "b": b2} for c in range(B)
    ]
    res = run_bass_kernel_spmd(nc, in_maps, core_ids=list(range(B)))
    return np.stack([res.results[c]["out"] for c in range(B)], axis=0)
